# revision 42
# baseline (speedup 1.0000x reference)
"""Trainium2 Bass kernel for GQA attention (B=2, S=2048, D=2048, 16 q-heads /
4 kv-heads, HD=128) with per-head QK RMSNorm + RoPE + causal softmax + output
projection.

Sharding: 8 cores = (batch b in {0,1}) x (kv-group g in {0..3}). Each core
computes its batch's 4 q-heads + 1 kv-head and a partial output through the
row-sharded Wo; the host sums the 4 partials per batch.

Attention computes scores TRANSPOSED (ST[k,t] = K_blk-stationary @ Q moving)
so exp(ST) is directly the moving operand of the PV matmul -- no probability
transposes. PT slabs are folded into an fp16 partial sum on the vector engine
(causally trimmed), so the softmax denominator costs one 512-col all-ones
matmul per (superblock, head), landing pre-broadcast across partitions for
the normalization multiply. Phase-1 rope math runs in fp16 on DVE 2x paths;
q/k head transposes run on the PE but are deferred one m-chunk so the PE
never waits on the norm/rope chain. Out-projection matmul groups are
sprinkled between attention chunks to fill exp-pipeline gaps.
"""
import numpy as np

import concourse.bass as bass  # noqa: F401
import concourse.mybir as mybir
import concourse.tile as tile
from concourse import bacc
from concourse.bass_utils import run_bass_kernel_spmd

F32 = mybir.dt.float32
F16 = mybir.dt.float16
AF = mybir.ActivationFunctionType
OP = mybir.AluOpType

B, S, D = 2, 2048, 2048
NH, NKV, HD = 16, 4, 128
REP = NH // NKV
EPS = 1e-6
EXPB = -5.0  # exp bias: cancels in softmax, keeps exp() in fp16 range


def build(s=S):
    """Build + compile the per-core SPMD program (identical on all 8 cores)."""
    sc = s // 128          # s-chunks
    kc = D // 128          # contraction chunks
    nsb = sc // 4          # q superblocks (512 wide)
    nc = bacc.Bacc("TRN2", target_bir_lowering=False, debug=False, num_devices=8)

    xby_d = nc.dram_tensor("xby", [sc, 128, kc * 128], F16, kind="ExternalInput")
    wqkv_d = nc.dram_tensor("wqkv", [D, 768], F16, kind="ExternalInput")
    wo_d = nc.dram_tensor("wo", [512, D], F16, kind="ExternalInput")
    ropes_d = nc.dram_tensor("ropes", [sc, 128, 512], F16, kind="ExternalInput")
    tri_d = nc.dram_tensor("trimask", [128, 128], F16, kind="ExternalInput")
    iden16_d = nc.dram_tensor("ident16", [128, 128], F16, kind="ExternalInput")
    out_d = nc.dram_tensor("outp", [s, D], F16, kind="ExternalOutput")

    with tile.TileContext(nc) as tc:
        with (
            tc.tile_pool(name="pers", bufs=1) as pers,
            tc.tile_pool(name="ptp", bufs=10) as ptp,
            tc.tile_pool(name="rcp", bufs=4) as rcp,
            tc.tile_pool(name="zap", bufs=8) as zap,
        ):
            qT = pers.tile([128, REP, s], F16, tag="qT")
            kT = pers.tile([128, s], F16, tag="kT")
            vv = pers.tile([128, sc, HD], F16, tag="vv")
            aoT = pers.tile([128, REP, s], F16, tag="aoT")
            tri_t = pers.tile([128, 128], F16, tag="trimask")
            wo_t = pers.tile([128, REP, D], F16, tag="wo")
            iden16_t = pers.tile([128, 128], F16, tag="ident16")
            eps_t = pers.tile([128, 1], F32, tag="eps")
            nc.vector.memset(eps_t[:], EPS)
            expb_t = pers.tile([128, 1], F32, tag="expb")
            nc.vector.memset(expb_t[:], EXPB)
            ones_t = pers.tile([128, 128], F16, tag="ones")
            nc.vector.memset(ones_t[:], 1.0)
            # f32 tile whose bit pattern is the rsqrt magic 0x5f3759df
            magic_t = pers.tile([128, 8], F32, tag="magic")
            nc.vector.memset(magic_t[:], 1.3211836172961054e19)

            # ---------------- Phase 1: QKV + RMSNorm + RoPE -----------------
            with (
                tc.tile_pool(name="wq", bufs=1) as wq,
                tc.tile_pool(name="xp", bufs=4) as xp,
                tc.tile_pool(name="cp", bufs=4) as cp,
                tc.tile_pool(name="st", bufs=3) as st,
                tc.tile_pool(name="p1q", bufs=2, space="PSUM") as p1q,
                tc.tile_pool(name="p1kv", bufs=2, space="PSUM") as p1kv,
                tc.tile_pool(name="p1t", bufs=2, space="PSUM") as p1t,
                tc.tile_pool(name="pA", bufs=1, space="PSUM") as pA,
                tc.tile_pool(name="pB", bufs=1, space="PSUM") as pB,
            ):
                wqkv_t = wq.tile([128, kc, 768], F16, tag="wqkv")
                wqkv_r = wqkv_d.rearrange("(dk ki) e -> ki dk e", ki=128)
                prev_qn = None

                # warm-up matmuls on already-memset SBUF data: keeps the PE
                # activity monitor fed during the initial DMA wait so the
                # first real matmuls run at full clock instead of 1.2 GHz
                wup = p1t.tile([128, 128], F32, tag="p1t", name="warmup")
                for _ in range(44):
                    nc.tensor.matmul(
                        wup[:], ones_t[:], ones_t[:],
                        start=True, stop=True,
                    )

                def emit_transposes(qn, kn, m):
                    pt = p1t.tile([128, 5, 128], F16, tag="p1t")
                    for h in range(REP):
                        nc.tensor.transpose(
                            pt[:, h], qn[:, h * 128:(h + 1) * 128], iden16_t[:],
                        )
                        if h < 2:
                            nc.vector.tensor_copy(
                                out=qT[:, h, m * 128:(m + 1) * 128], in_=pt[:, h],
                            )
                        else:
                            nc.scalar.copy(
                                out=qT[:, h, m * 128:(m + 1) * 128], in_=pt[:, h],
                            )
                    nc.tensor.transpose(pt[:, 4], kn[:], iden16_t[:])
                    nc.scalar.copy(
                        out=kT[:, m * 128:(m + 1) * 128], in_=pt[:, 4],
                    )

                def p1_post(m, pq, pkv, cst):
                    # evict to fp16 + RMSNorm + RoPE for one m-chunk; the
                    # transposes are deferred (queued on prev_q) so the PE
                    # never waits on this chain.  The q rope constants are
                    # stored once (128 cols) and broadcast across the 4
                    # heads with stride-0 APs instead of being tiled 4x in
                    # DRAM (cuts the ropes stream from 1280 to 512 cols)
                    cqw = cst[:, 0:128].unsqueeze(1).broadcast_to([128, 4, 128])
                    sqw = cst[:, 128:256]
                    ckw = cst[:, 256:384]
                    skw = cst[:, 384:512]

                    # ---- evict to fp16 (scalar engine) ----
                    pq16 = st.tile([128, 512], F16, tag="pq16")
                    nc.scalar.copy(out=pq16[:], in_=pq)
                    pk16 = st.tile([128, 128], F16, tag="pk16")
                    nc.scalar.copy(out=pk16[:], in_=pkv[:, 0:128])
                    nc.scalar.copy(out=vv[:, m, :], in_=pkv[:, 128:256])

                    # ---- RMSNorm stats: squares + reduce ----
                    sq16 = st.tile([128, 512], F16, tag="sq16")
                    nc.vector.tensor_mul(sq16[:], pq16[:], pq16[:])
                    sk16 = st.tile([128, 128], F16, tag="sk16")
                    nc.vector.tensor_mul(sk16[:], pk16[:], pk16[:])
                    ss = st.tile([128, 16], F32, tag="ss")
                    nc.vector.tensor_reduce(
                        out=ss[:, 0:4],
                        in_=sq16[:].rearrange("p (h d) -> p h d", d=128),
                        axis=mybir.AxisListType.X, op=OP.add,
                    )
                    nc.vector.tensor_reduce(
                        out=ss[:, 4:5],
                        in_=sk16[:].rearrange("p (o d) -> p o d", d=128),
                        axis=mybir.AxisListType.X, op=OP.add,
                    )
                    # rsqrt(var+eps) entirely on the vector engine (magic
                    # constant + 2 Newton steps): keeps the ACT engine's
                    # table set on exp for the whole kernel (a Sqrt here
                    # would force a ~1.3us table switch at every use)
                    vn = st.tile([128, 8], F32, tag="vn")
                    nc.vector.tensor_scalar(
                        out=vn[:, 0:5], in0=ss[:, 0:5],
                        scalar1=1.0 / HD, scalar2=EPS,
                        op0=OP.mult, op1=OP.add,
                    )
                    sh = st.tile([128, 8], mybir.dt.int32, tag="sh")
                    nc.vector.tensor_scalar(
                        out=sh[:, 0:5], in0=vn[:, 0:5].bitcast(mybir.dt.int32),
                        scalar1=1, scalar2=None,
                        op0=OP.logical_shift_right,
                    )
                    y0 = st.tile([128, 8], F32, tag="y0")
                    nc.vector.tensor_sub(
                        y0[:, 0:5].bitcast(mybir.dt.int32),
                        magic_t[:, 0:5].bitcast(mybir.dt.int32),
                        sh[:, 0:5],
                    )
                    vh = st.tile([128, 8], F32, tag="vh")
                    nc.vector.tensor_scalar_mul(vh[:, 0:5], vn[:, 0:5], 0.5)
                    rs = st.tile([128, 8], F32, tag="rs")
                    cur = y0
                    for it in range(2):
                        aa = st.tile([128, 8], F32, tag="nta", name=f"nta{m}_{it}")
                        nc.vector.tensor_mul(aa[:, 0:5], cur[:, 0:5], cur[:, 0:5])
                        nc.vector.tensor_mul(aa[:, 0:5], aa[:, 0:5], vh[:, 0:5])
                        nc.vector.tensor_scalar(
                            out=aa[:, 0:5], in0=aa[:, 0:5],
                            scalar1=-1.0, scalar2=1.5,
                            op0=OP.mult, op1=OP.add,
                        )
                        nxt = rs if it == 1 else st.tile(
                            [128, 8], F32, tag="ntb", name=f"ntb{m}",
                        )
                        nc.vector.tensor_mul(nxt[:, 0:5], cur[:, 0:5], aa[:, 0:5])
                        cur = nxt

                    # ---- RoPE in fp16 (DVE 2x) ----
                    ra = st.tile([128, 512], F16, tag="ra")
                    ra3 = ra[:].rearrange("p (h d) -> p h d", d=128)
                    pq3 = pq16[:].rearrange("p (h d) -> p h d", d=128)
                    nc.vector.tensor_mul(ra3, pq3, cqw)
                    rb = st.tile([128, 512], F16, tag="rb")
                    rb3 = rb[:].rearrange("p (h u d) -> p h u d", u=2, d=64)
                    pq4 = pq16[:].rearrange("p (h u d) -> p h u d", u=2, d=64)
                    sq3 = sqw.rearrange("p (u d) -> p u d", u=2, d=64) \
                        .unsqueeze(1).broadcast_to([128, 4, 2, 64])
                    nc.vector.tensor_mul(rb3[:, :, 0], pq4[:, :, 1], sq3[:, :, 0])
                    nc.vector.tensor_mul(rb3[:, :, 1], pq4[:, :, 0], sq3[:, :, 1])
                    qn0 = st.tile([128, 512], F16, tag="qn0")
                    nc.vector.tensor_add(qn0[:], ra[:], rb[:])
                    qn = st.tile([128, 512], F16, tag="qn")
                    for h in range(REP):
                        nc.vector.tensor_scalar_mul(
                            qn[:, h * 128:(h + 1) * 128],
                            qn0[:, h * 128:(h + 1) * 128],
                            rs[:, h:h + 1],
                        )

                    # ---- k head rope ----
                    rak = st.tile([128, 128], F16, tag="rak")
                    nc.vector.tensor_mul(rak[:], pk16[:], ckw)
                    rbk = st.tile([128, 128], F16, tag="rbk")
                    nc.vector.tensor_mul(rbk[:, 0:64], pk16[:, 64:128], skw[:, 0:64])
                    nc.vector.tensor_mul(rbk[:, 64:128], pk16[:, 0:64], skw[:, 64:128])
                    kn0 = st.tile([128, 128], F16, tag="kn0")
                    nc.vector.tensor_add(kn0[:], rak[:], rbk[:])
                    kn = st.tile([128, 128], F16, tag="kn")
                    nc.vector.tensor_scalar_mul(kn[:], kn0[:], rs[:, 4:5])
                    prev_q.append((qn, kn, m))

                prev_q = []

                # ---- superblock Q=0 attention, run as 20 small units
                # sprinkled into phase-1's idle slots (m=6..15): it only
                # needs rows 0-511 of qT/kT/vv, ready by m=5.  Uses two
                # spare PSUM banks: pA rotates ST tiles and the Z matmul,
                # pB holds one head's PV accumulator.  All exps here reuse
                # the exp table set (no Sqrt on ACT anywhere -> no table
                # thrash).
                b0 = {}
                b0_units = []

                def _b0_pv(h, c, sl):
                    j = 2 * c + sl
                    c0 = j * 128 if j > 0 else 0
                    ptc = b0[("pt", h, c)]
                    nc.tensor.matmul(
                        b0["pvp"][:, c0:], vv[:, j, :], ptc[:, sl, c0:],
                        start=(j == 0), stop=(j == 3),
                        skip_group_check=True,
                    )

                def _b0_st(h, c, sl):
                    j = 2 * c + sl
                    c0 = j * 128 if j > 0 else 0
                    if sl == 0:
                        ptc = ptp.tile([128, 2, 512], F16, tag="ptc",
                                       name=f"b0p{h}_{c}")
                        b0[("pt", h, c)] = ptc
                    ptc = b0[("pt", h, c)]
                    stc = pA.tile([128, 512], F32, tag="pA",
                                  name=f"b0s{h}_{c}_{sl}")
                    nc.tensor.matmul(
                        stc[:, c0:],
                        kT[:, j * 128:(j + 1) * 128],
                        qT[:, h, c0:512],
                        start=True, stop=True,
                    )
                    nc.scalar.activation(
                        ptc[:, sl, c0:], stc[:, c0:], AF.Exp, bias=expb_t[:],
                    )
                    blk = ptc[:, sl, j * 128:(j + 1) * 128]
                    nc.vector.tensor_mul(blk, blk, tri_t[:])
                    if j == 0:
                        za = zap.tile([128, 512], F16, tag="zacc",
                                      name=f"b0z{h}")
                        b0["za"] = za
                        nc.vector.tensor_copy(out=za[:], in_=ptc[:, 0, :])
                    else:
                        za = b0["za"]
                        nc.vector.tensor_add(
                            za[:, c0:], za[:, c0:], ptc[:, sl, c0:],
                        )

                def _mk_b0_unit(h, idx):
                    def unit():
                        if idx == 0:
                            b0["pvp"] = pB.tile([128, 512], F32, tag="pB",
                                                name=f"b0v{h}")
                            _b0_st(h, 0, 0)
                        elif idx == 1:
                            _b0_pv(h, 0, 0)
                            _b0_st(h, 0, 1)
                        elif idx == 2:
                            _b0_pv(h, 0, 1)
                            _b0_st(h, 1, 0)
                        elif idx == 3:
                            _b0_pv(h, 1, 0)
                            _b0_st(h, 1, 1)
                        else:
                            _b0_pv(h, 1, 1)
                            zb = pA.tile([128, 512], F32, tag="pA",
                                         name=f"b0zb{h}")
                            nc.tensor.matmul(zb[:], ones_t[:], b0["za"][:],
                                             start=True, stop=True)
                            rec = rcp.tile([128, 512], F32, tag="rec",
                                           name=f"b0r{h}")
                            nc.vector.reciprocal_approx_fast(
                                out=rec[:], in_=zb[:],
                            )
                            nc.vector.tensor_mul(
                                aoT[:, h, 0:512], b0["pvp"][:], rec[:],
                            )
                    b0_units.append(unit)

                for _h in range(REP):
                    for _i in range(5):
                        _mk_b0_unit(_h, _i)

                # ---- m=0 and m=1 fused, k-interleaved with the JIT weight
                # stream: both chunks' matmuls for contraction chunk k run as
                # soon as wqkv chunk k lands, so the PE tracks the DMA stream
                # (~0.64us of matmul per ~0.65us weight-chunk transfer)
                xt0 = xp.tile([128, kc * 128], F16, tag="xt", name="xt0")
                xt1 = xp.tile([128, kc * 128], F16, tag="xt", name="xt1")
                cst0 = cp.tile([128, 512], F16, tag="cst", name="cst0")
                cst1 = cp.tile([128, 512], F16, tag="cst", name="cst1")
                for k in range(kc):
                    if k % 4 == 0:
                        nc.sync.dma_start(
                            out=xt0[:, k * 128:(k + 4) * 128],
                            in_=xby_d[0, :, k * 128:(k + 4) * 128],
                        )
                    if k % 4 == 2:
                        p = (k - 2) // 4
                        nc.sync.dma_start(
                            out=xt1[:, p * 512:(p + 1) * 512],
                            in_=xby_d[1, :, p * 512:(p + 1) * 512],
                        )
                    eng = nc.sync if k % 2 == 0 else nc.gpsimd
                    eng.dma_start(out=wqkv_t[:, k], in_=wqkv_r[:, k])
                nc.gpsimd.dma_start(out=iden16_t[:], in_=iden16_d[:, :])
                nc.gpsimd.dma_start(out=tri_t[:], in_=tri_d[:, :])
                # m=2's x tile goes ahead of the rope constants on sync: its
                # matmuls are the first PE-blocking consumer after the fused
                # ramp, while the ropes only feed the (deferred) norm chain
                xt2 = xp.tile([128, kc * 128], F16, tag="xt", name="xt2")
                nc.sync.dma_start(out=xt2[:], in_=xby_d[2])
                nc.sync.dma_start(out=cst0[:], in_=ropes_d[0])
                nc.sync.dma_start(out=cst1[:], in_=ropes_d[1])

                x03 = xt0[:].rearrange("p (dk t) -> p dk t", t=128)
                x13 = xt1[:].rearrange("p (dk t) -> p dk t", t=128)
                pq0 = p1q.tile([128, 512], F32, tag="p1q", name="pq0")
                pq1 = p1q.tile([128, 512], F32, tag="p1q", name="pq1")
                pkv0 = p1kv.tile([128, 512], F32, tag="p1kv", name="pkv0")
                pkv1 = p1kv.tile([128, 512], F32, tag="p1kv", name="pkv1")
                for k in range(kc):
                    for pq, pkv, x3 in ((pq0, pkv0, x03), (pq1, pkv1, x13)):
                        nc.tensor.matmul(
                            pq, x3[:, k], wqkv_t[:, k, 0:512],
                            start=(k == 0), stop=(k == kc - 1),
                            skip_group_check=True,
                        )
                        nc.tensor.matmul(
                            pkv[:, 0:256], x3[:, k], wqkv_t[:, k, 512:768],
                            start=(k == 0), stop=(k == kc - 1),
                            skip_group_check=True,
                        )
                p1_post(0, pq0, pkv0, cst0)
                p1_post(1, pq1, pkv1, cst1)

                for m in range(2, sc):
                    # prefetch DMAs first so they issue ahead of dependent work
                    cst = cp.tile([128, 512], F16, tag="cst")
                    if m == 2:
                        xt = xt2
                    else:
                        xt = xp.tile([128, kc * 128], F16, tag="xt")
                        nc.sync.dma_start(out=xt[:], in_=xby_d[m])
                    nc.sync.dma_start(out=cst[:], in_=ropes_d[m])
                    if m == 6:
                        # wo (2MB) is not needed until the first outproj
                        # group in phase 2; loading it during the ramp
                        # starves the m2/m3 x prefetches
                        nc.gpsimd.dma_start(
                            out=wo_t[:],
                            in_=wo_d.rearrange("(e ki) d -> ki e d", ki=128),
                        )

                    xt3 = xt[:].rearrange("p (dk t) -> p dk t", t=128)
                    pq = p1q.tile([128, 512], F32, tag="p1q")
                    pkv = p1kv.tile([128, 512], F32, tag="p1kv")
                    for k in range(kc):
                        nc.tensor.matmul(
                            pq, xt3[:, k], wqkv_t[:, k, 0:512],
                            start=(k == 0), stop=(k == kc - 1),
                        )
                    if m >= 6 and b0_units:
                        b0_units.pop(0)()
                    for k in range(kc):
                        nc.tensor.matmul(
                            pkv[:, 0:256], xt3[:, k], wqkv_t[:, k, 512:768],
                            start=(k == 0), stop=(k == kc - 1),
                        )
                    emit_transposes(*prev_q.pop(0))
                    if m >= 5 and b0_units:
                        b0_units.pop(0)()
                    p1_post(m, pq, pkv, cst)
                while prev_q:
                    emit_transposes(*prev_q.pop(0))

            # ------- Phase 2+3: causal attention + fused out-projection -----
            with (
                tc.tile_pool(name="ob", bufs=3) as ob,
                tc.tile_pool(name="psST", bufs=2, space="PSUM") as psST,  # 2x2 banks
                tc.tile_pool(name="psV", bufs=2, space="PSUM") as psV,    # 2 banks
                tc.tile_pool(name="psPO", bufs=2, space="PSUM") as psPO,  # 2 banks
            ):
                pending = []   # outproj (m, n) groups not yet emitted
                otmap = {}
                popctl = [0.0, 0.0]   # [accumulator, pops-per-slot rate]

                def emit_po_group():
                    m, n = pending.pop(0)
                    if n == 0:
                        otmap[m] = ob.tile([128, D], F16, tag="ot", name=f"ot{m}")
                    po = psPO.tile([128, 512], F32, tag="psPO")
                    for e in range(REP):
                        nc.tensor.matmul(
                            po[:], aoT[:, e, m * 128:(m + 1) * 128],
                            wo_t[:, e, n * 512:(n + 1) * 512],
                            start=(e == 0), stop=(e == REP - 1),
                        )
                    # split evictions across vector/scalar so neither engine
                    # becomes the phase-2 pacer
                    if n % 2 == 0:
                        nc.vector.tensor_copy(
                            out=otmap[m][:, n * 512:(n + 1) * 512], in_=po[:],
                        )
                    else:
                        nc.scalar.copy(
                            out=otmap[m][:, n * 512:(n + 1) * 512], in_=po[:],
                        )
                    if n % 2 == 1:
                        oeng = nc.sync if n == 1 else nc.gpsimd
                        oeng.dma_start(
                            out=out_d[m * 128:(m + 1) * 128,
                                      (n - 1) * 512:(n + 1) * 512],
                            in_=otmap[m][:, (n - 1) * 512:(n + 1) * 512],
                        )
                    if n == 3:
                        del otmap[m]

                def sprinkle():
                    if pending:
                        popctl[0] += popctl[1]
                        while popctl[0] >= 1.0 and pending:
                            emit_po_group()
                            popctl[0] -= 1.0

                def attention_pair(Q, hA, hB):
                    # two heads' chunk streams interleaved so each head's
                    # exp latency is hidden behind the other head's ST/PV
                    # matmuls; all slabs (incl. masked diagonal) fold into
                    # two fp16 parity accumulators so the softmax denominator
                    # is a single 512-col ones-matmul per head
                    jlast = 4 * Q + 3
                    nch = 2 * (Q + 1)
                    heads = (hA, hB)
                    # seed one extra filler group into the first sprinkle
                    # slot: the pair's first PVs wait a full exp latency
                    # with nothing else queued (borrowed from this block's
                    # budget, so the total drained is unchanged)
                    popctl[0] += 1.0
                    pvp = {}
                    for h in heads:
                        pvp[h] = psV.tile([128, 512], F32, tag="psV",
                                          name=f"pvp{Q}_{h}")
                    pts = {}
                    zacc = {}

                    def emit_st(h, c):
                        stc = psST.tile([128, 2, 512], F32, tag="psST")
                        for sl in range(2):
                            j = 2 * c + sl
                            jj = j - 4 * Q
                            c0 = jj * 128 if jj > 0 else 0
                            nc.tensor.matmul(
                                stc[:, sl, c0:],
                                kT[:, j * 128:(j + 1) * 128],
                                qT[:, h, Q * 512 + c0:(Q + 1) * 512],
                                start=True, stop=True,
                            )
                        ptc = ptp.tile([128, 2, 512], F16, tag="ptc")
                        pts[(h, c)] = ptc
                        nc.scalar.activation(ptc[:], stc[:], AF.Exp, bias=expb_t[:])
                        diag = c >= nch - 2
                        if diag:
                            # causal triangle on the diagonal 128-blocks
                            for sl in range(2):
                                jj = 2 * c + sl - 4 * Q
                                blk = ptc[:, sl, jj * 128:(jj + 1) * 128]
                                nc.vector.tensor_mul(blk, blk, tri_t[:])
                        # fold into a parity accumulator; diagonal slabs only
                        # contribute their written region [c0:] (the rest of
                        # the PSUM tile is stale garbage from prior chunks)
                        par = 0 if Q == 0 else c % 2
                        za = zacc.get((h, par))
                        if not diag:
                            if za is None:
                                za = zap.tile([128, 512], F16, tag="zacc",
                                              name=f"za{Q}_{h}_{par}")
                                zacc[(h, par)] = za
                                nc.vector.tensor_add(
                                    za[:], ptc[:, 0, :], ptc[:, 1, :],
                                )
                            else:
                                nc.vector.tensor_add(za[:], za[:], ptc[:, 0, :])
                                nc.vector.tensor_add(za[:], za[:], ptc[:, 1, :])
                        else:
                            for sl in range(2):
                                jj = 2 * c + sl - 4 * Q
                                c0 = jj * 128 if jj > 0 else 0
                                if za is None:  # only Q==0, c==0, sl==0
                                    za = zap.tile([128, 512], F16, tag="zacc",
                                                  name=f"za{Q}_{h}_{par}")
                                    zacc[(h, par)] = za
                                    nc.vector.tensor_copy(
                                        out=za[:], in_=ptc[:, 0, :],
                                    )
                                else:
                                    nc.vector.tensor_add(
                                        za[:, c0:], za[:, c0:],
                                        ptc[:, sl, c0:],
                                    )

                    def emit_pv(h, c):
                        ptc = pts.pop((h, c))
                        for sl in range(2):
                            j = 2 * c + sl
                            jj = j - 4 * Q
                            c0 = jj * 128 if jj > 0 else 0
                            nc.tensor.matmul(
                                pvp[h][:, c0:], vv[:, j, :], ptc[:, sl, c0:],
                                start=(j == 0), stop=(j == jlast),
                                skip_group_check=True,
                            )

                    for c in range(nch):
                        for h in heads:
                            emit_st(h, c)
                        for h in heads:
                            if c > 0:
                                emit_pv(h, c - 1)
                            sprinkle()
                    for h in heads:
                        emit_pv(h, nch - 1)
                        sprinkle()
                    zbs = {}
                    for h in heads:
                        za = zacc[(h, 0)]
                        if (h, 1) in zacc:
                            nc.vector.tensor_add(za[:], za[:], zacc[(h, 1)][:])
                        zb = psPO.tile([128, 512], F32, tag="psPO")
                        nc.tensor.matmul(zb[:], ones_t[:], za[:],
                                         start=True, stop=True)
                        zbs[h] = zb
                    for h in heads:
                        rec = rcp.tile([128, 512], F32, tag="rec")
                        nc.vector.reciprocal_approx_fast(out=rec[:], in_=zbs[h][:])
                        nc.vector.tensor_mul(
                            aoT[:, h, Q * 512:(Q + 1) * 512], pvp[h][:], rec[:],
                        )

                # Q=0 already ran inside phase 1; its outproj groups are
                # available immediately as filler for the first block here.
                # On the last block, hold back a few filler groups so the
                # PE has work during the final normalize-chain handoff
                # before the tail drain.
                for m_ in range(0, 4):
                    for n_ in range(D // 512):
                        pending.append((m_, n_))
                order = [1, nsb - 1, nsb - 2]
                for qi, Q in enumerate(order):
                    nch = 2 * (Q + 1)
                    hold = 3 if qi == len(order) - 1 else 0
                    popctl[0] = 0.0
                    popctl[1] = max(0, len(pending) - hold) / (4.0 * nch)
                    attention_pair(Q, 0, 1)
                    attention_pair(Q, 2, 3)
                    for m in range(4 * Q, 4 * Q + 4):
                        for n in range(D // 512):
                            pending.append((m, n))
                while pending:
                    emit_po_group()

    nc.compile()
    return nc


def make_in_maps(x, cos, sin, Wq, Wk, Wv, Wo, q_norm_w, k_norm_w):
    qsc = (q_norm_w / np.sqrt(HD)).astype(np.float32)
    ksc = k_norm_w.astype(np.float32)

    def rope_consts(w):
        cw = (cos * w[None, :]).astype(np.float32)
        sw = np.empty_like(cw)
        sw[:, :64] = -sin[:, :64] * w[None, 64:]
        sw[:, 64:] = sin[:, 64:] * w[None, :64]
        return cw, sw

    cwq, swq = rope_consts(qsc)
    cwk, swk = rope_consts(ksc)
    ropes = np.concatenate([cwq, swq, cwk, swk], axis=1).astype(np.float16)
    ropes = np.ascontiguousarray(ropes.reshape(S // 128, 128, 512))
    r = np.arange(128)
    # trimask[k, t] = 1 where t >= k (valid causal), else 0
    tri = (r[None, :] >= r[:, None]).astype(np.float16)
    ident16 = np.eye(128, dtype=np.float16)

    in_maps = []
    for c in range(8):
        b, g = c // 4, c % 4
        # xby[m, ki, dk, t] = x[b, m*128 + t, dk*128 + ki]
        xby = np.ascontiguousarray(
            x[b].reshape(S // 128, 128, D // 128, 128).transpose(0, 3, 2, 1)
            .reshape(S // 128, 128, D).astype(np.float16)
        )
        wqkv = np.ascontiguousarray(
            np.concatenate(
                [
                    Wq[:, g * 512:(g + 1) * 512],
                    Wk[:, g * 128:(g + 1) * 128],
                    Wv[:, g * 128:(g + 1) * 128],
                ],
                axis=1,
            ).astype(np.float16)
        )
        wo = np.ascontiguousarray(Wo[g * 512:(g + 1) * 512, :].astype(np.float16))
        in_maps.append(
            dict(
                xby=xby, wqkv=wqkv, wo=wo, ropes=ropes, trimask=tri,
                ident16=ident16,
            )
        )
    return in_maps


_cached = None


def kernel(x, cos, sin, Wq, Wk, Wv, Wo, q_norm_w, k_norm_w):
    global _cached
    x = np.asarray(x, np.float32)
    cos = np.asarray(cos, np.float32)
    sin = np.asarray(sin, np.float32)
    in_maps = make_in_maps(
        x, cos, sin,
        np.asarray(Wq, np.float32), np.asarray(Wk, np.float32),
        np.asarray(Wv, np.float32), np.asarray(Wo, np.float32),
        np.asarray(q_norm_w, np.float32), np.asarray(k_norm_w, np.float32),
    )
    if _cached is None:
        _cached = build()
    res = run_bass_kernel_spmd(_cached, in_maps, core_ids=list(range(8)))
    out = np.zeros((B, S, D), np.float64)
    for c in range(8):
        out[c // 4] += res.results[c]["outp"].astype(np.float64)
    return out.astype(np.float32)

